# revision 34
# baseline (speedup 1.0000x reference)
"""Trainium2 Bass kernel for nn_DirectedHyperConvLayer (GNN message passing).

Self-contained: accepts FULL inputs, shards across 8 NeuronCores internally,
returns the FULL [50000, 64] float32 output.

Sharding: each core owns a contiguous block of destination rows; the host
routes/sorts edges by destination row (and by source row within each group
for gather locality), pads them into 128-edge tiles grouped by 128-row
destination windows, and splits each window's edges into low/high passes so
source indices fit dma_gather's int16 limit. On device, combined
[raw_bf16 | normalized_bf16] rows are fetched with batched dma_gather calls;
per-edge cosine weights use a host-supplied f8 one-hot selecting the
destination-window embedding block through the tensor engine; weighted
messages are segment-summed by one-hot matmuls into PSUM per window.

v2: the normalized-embedding table (aug) is host-precomputed and staged as a
replicated input (no prologue normalize, no first AllGather); spr one-hots
stream in a few large chunked DMAs instead of per-window transfers; stage
outputs accumulate in an SBUF staging tile (partition-major layout) and are
written with one large DMA per stage; only the inter-hop AllGather remains.
"""


import math
from dataclasses import dataclass

import numpy as np
import ml_dtypes

import concourse.bass as bass
import concourse.bacc as bacc
import concourse.mybir as mybir
import concourse.tile as tile

F32 = mybir.dt.float32
BF16 = mybir.dt.bfloat16
F8 = mybir.dt.float8e4
I16 = mybir.dt.int16
I32 = mybir.dt.int32
NP_F8 = mybir.dt.np(F8)
NP_BF16 = mybir.dt.np(BF16)
P = 128
TB = 16  # tiles per compute batch
TBG = 8  # tiles per gather sub-call (1024 idx — proven ucode granularity)
HB = 16  # tiles per compute half-batch (must divide TB)
CT = 112  # spr tiles per streamed chunk
ALPHA = 0.1
REDUCE_ON_POOL = False  # gpsimd can't do free-axis reduce (C-axis only)


@dataclass
class Config:
    n_nodes: int = 50000
    d: int = 64
    n_cores: int = 8
    rpc: int = 6272  # rows per core (multiple of 128)
    split: int = 32768  # low/high gather split (<= 32768)

    @property
    def nw(self):
        return self.rpc // P

    @property
    def n_pad(self):
        return self.n_cores * self.rpc

    @property
    def nblk(self):
        return self.n_pad // P


@dataclass
class StageSched:
    T: np.ndarray  # [2, nw] tiles per (pass, window)
    n_tiles: tuple  # (low, high) tile counts (each % TB == 0)

    @property
    def total_tiles(self):
        return int(self.n_tiles[0] + self.n_tiles[1])

    def tile_windows(self):
        """list over global tile index -> (pass, w, j_in_window, first, last)"""
        out = []
        for p in range(2):
            for w in range(self.T.shape[1]):
                Tw = int(self.T[p, w])
                for j in range(Tw):
                    out.append((p, w, j, j == 0, j == Tw - 1))
        return out


def route_edges(cfg: Config, edge_index, edge_val, hi_map=None, col_map=None):
    """Returns (sched, per_core list of dicts with idx16/val05/dloc/spr).

    hi_map/col_map: optional [n_pad] arrays giving, per original source row,
    which gather table it lives in (0/1) and its row index there. Defaults
    describe the stage-1 aug table split at cfg.split."""
    r0 = np.asarray(edge_index[0], dtype=np.int64)
    r1 = np.asarray(edge_index[1], dtype=np.int64)
    val = np.asarray(edge_val, dtype=np.float32)
    E = r0.shape[0]
    nc_, nw = cfg.n_cores, cfg.nw

    k = r0 // cfg.rpc
    w = (r0 % cfg.rpc) // P
    dloc = r0 % P
    if hi_map is None:
        hi = (r1 >= cfg.split).astype(np.int64)
        col = r1 - hi * cfg.split
    else:
        hi = hi_map[r1]
        col = col_map[r1]
    gid = (k * 2 + hi) * nw + w

    counts = np.bincount(gid, minlength=nc_ * 2 * nw).reshape(nc_, 2, nw)
    T = np.ceil(counts.max(axis=0) / P).astype(np.int64)  # [2, nw]
    # pad each pass's tile total to a multiple of TB
    for pss in range(2):
        T[pss, nw - 1] += (-int(T[pss].sum())) % TB
    nt_low, nt_high = int(T[0].sum()), int(T[1].sum())
    n_tiles = nt_low + nt_high
    # global tile base per (pass, w)
    tbase = np.zeros((2, nw), dtype=np.int64)
    tbase[0] = np.cumsum(T[0]) - T[0]
    tbase[1] = nt_low + np.cumsum(T[1]) - T[1]

    # slot within (k, hi, w) group; edges sorted by source row within each
    # group so gather descriptors hit ascending HBM addresses
    order = np.argsort(gid * (1 << 17) + col, kind="stable")
    sorted_gid = gid[order]
    starts = np.searchsorted(sorted_gid, np.arange(nc_ * 2 * nw))
    ranks = np.empty(E, dtype=np.int64)
    ranks[order] = np.arange(E) - starts[sorted_gid]

    tile_g = tbase[hi, w] + ranks // P  # global tile per edge
    pos = ranks % P
    idx_val = col.astype(np.int16)

    per_core = []
    for kk in range(nc_):
        m = k == kk
        tg, pg = tile_g[m], pos[m]
        slots = tg * P + pg
        # padding slots gather row 0 (mid-call negative indices crash the
        # gather ucode); their val=0 and dloc=-1 zero their contribution
        idx_flat = np.zeros(n_tiles * P, dtype=np.int16)
        val_flat = np.zeros(n_tiles * P, dtype=np.float32)
        dloc_flat = np.full(n_tiles * P, -1.0, dtype=np.float32)
        idx_flat[slots] = idx_val[m]
        val_flat[slots] = 0.05 * val[m]
        dloc_flat[slots] = dloc[m].astype(np.float32)
        # idx16 wrapped per 1024-idx gather sub-call: [128, (n_tiles//TBG)*cwg]
        cwg = TBG * P // 16
        ncalls_g = n_tiles // TBG
        iw = idx_flat.reshape(ncalls_g, cwg, 16)
        iw = np.transpose(iw, (2, 0, 1)).reshape(16, ncalls_g * cwg)
        idx16 = np.tile(iw, (8, 1))
        # val/dloc [128, n_tiles] (partition p, tile t)
        val2d = val_flat.reshape(n_tiles, P).T.copy()
        dloc2d = dloc_flat.reshape(n_tiles, P).T.astype(NP_BF16)
        # S' [128(d), n_tiles*128(e)] f8
        spr = np.zeros((P, n_tiles * P), dtype=NP_F8)
        spr[dloc[m], slots] = NP_F8(1.0)
        per_core.append(
            {"idx": idx16, "val": val2d, "dloc": dloc2d, "spr": spr}
        )
    return StageSched(T=T, n_tiles=(nt_low, nt_high)), per_core


def _emit_stage(
    tc, cfg, sched: StageSched, pools, consts, tables, idx_t, val_t, dloc_t,
    spr_dram, out_close, on_window_close=None, mid_hooks=None,
):
    """Emit one spmm stage. out_close(w, psum_ap, acc_ap, has_low) writes the
    finished window; on_window_close(w) fires after each window close;
    mid_hooks[c] is invoked at the top of compute batch c."""
    nc = tc.nc
    nw = cfg.nw
    d = cfg.d
    gp, aalp, paccp, dvep, msgp, s8p, accp, sprp = (
        pools["g"], pools["aal"], pools["pacc"], pools["dve"], pools["msg"],
        pools["s8"], pools["acc"], pools["spr"],
    )
    strip = consts["strip"]
    iota_mid = consts["iota_mid"]

    acc = accp.tile([P, nw * d], F32, tag="acc")
    nc.vector.memset(acc[:], 0.0)

    def close_window(w, psum_ap, acc_ap, has_low):
        out_close(w, psum_ap, acc_ap, has_low)
        if on_window_close is not None:
            on_window_close(w)

    # windows with no edges at all: close immediately from the zeroed acc
    for w in range(nw):
        if sched.T[0, w] == 0 and sched.T[1, w] == 0:
            close_window(w, None, acc[:, w * d : (w + 1) * d], False)

    tw = sched.tile_windows()
    n_tiles = sched.total_tiles
    assert n_tiles % TB == 0
    ncalls = n_tiles // TB
    nt_low = sched.n_tiles[0]
    cwg = TBG * P // 16  # idx cols per gather sub-call
    nsub = TB // TBG  # gather sub-calls per compute batch

    # spr streamed in CT-tile chunks (few large DMAs, rolling triple-buffer)
    nchunks = (n_tiles + CT - 1) // CT
    spr_chunks = {}
    next_chunk = [0]

    def issue_chunk():
        ci = next_chunk[0]
        t0 = ci * CT
        ctn = min(CT, n_tiles - t0)
        st = sprp.tile([P, CT * P], F8, tag="spr", name="spr")
        nc.scalar.dma_start(st[:, 0 : ctn * P], spr_dram[:, t0 * P : (t0 + ctn) * P])
        spr_chunks[ci] = st
        next_chunk[0] += 1

    while next_chunk[0] < min(2, nchunks):
        issue_chunk()

    win_psum = {}
    g_tiles = {}
    # exactly 2 outstanding gather calls per SWDGE queue (8 sub-calls / 4
    # queues): the ring holds no more — deeper lookahead crashes the ucode.
    LOOKAHEAD = 4

    gseen = consts.setdefault("gseen", [0])

    def issue_gather(c):
        pss = 0 if c * TB < nt_low else 1
        tab = tables[pss]
        g = gp.tile([P, TB, 2 * d], BF16, tag="g")
        if gseen[0] < gp.bufs:
            # first touch of this ring buffer: clear it so slots skipped by
            # negative gather indices hold finite values, not random bits
            nc.vector.memset(g[:], 0.0)
            gseen[0] += 1
        for sub in range(nsub):
            sc = c * nsub + sub  # global gather sub-call index
            nc.gpsimd.dma_gather(
                out_ap=g[:, sub * TBG : (sub + 1) * TBG, :],
                in_ap=tab,
                idxs_ap=idx_t[:, sc * cwg : (sc + 1) * cwg],
                num_idxs=TBG * P,
                num_idxs_reg=TBG * P,
                elem_size=2 * d,
                queue_num=sc % 4,
                single_packet=False,
            )
        g_tiles[c] = g

    for c in range(min(LOOKAHEAD, ncalls)):
        issue_gather(c)
    for c in range(ncalls):
        if mid_hooks and c in mid_hooks:
            mid_hooks[c]()
        # keep 2 spr chunks of lookahead in flight
        while next_chunk[0] < nchunks and next_chunk[0] * CT < (c + 3) * TB:
            issue_chunk()
        g = g_tiles.pop(c)
        for half in range(TB // HB):
            h = c * (TB // HB) + half  # half-batch index
            t0 = c * TB + half * HB  # first tile of this half-batch
            # one-hot S8 [128(e), HB, 128(d)] in bf16 — emitted first so DVE
            # computes it while the batch's gather is still in flight
            s8 = s8p.tile([P, HB * P], BF16, tag="s8")
            s8v = bass.AP(
                s8[:].tensor, s8[:].offset, [s8[:].ap[0], [P, HB], [1, P]]
            )
            nc.vector.tensor_tensor(
                out=s8v,
                in0=dloc_t[:, t0 : t0 + HB].to_broadcast([P, HB, P]),
                in1=iota_mid(HB),
                op=mybir.AluOpType.is_equal,
            )
            # Aal matmuls (per tile)
            aal = aalp.tile([P, HB, d], F32, space="PSUM", tag="aal")
            for sl in range(HB):
                t = t0 + sl
                pss_t, w, _, _, _ = tw[t]
                st = spr_chunks[t // CT]
                jj = t % CT
                nc.tensor.matmul(
                    out=aal[:, sl, :],
                    lhsT=st[:, jj * P : (jj + 1) * P],
                    rhs=strip[:, w * d : (w + 1) * d],
                    start=True,
                    stop=True,
                )
            gsl = g[:, half * HB : (half + 1) * HB, :]
            # drain aal PSUM -> SBUF bf16 on ACT so DVE runs in 16-bit mode
            aals = dvep.tile([P, HB, d], BF16, tag="aals")
            nc.scalar.copy(out=aals[:], in_=aal[:])
            # dot / weight / msgs (batched over HB tiles)
            prod = dvep.tile([P, HB, d], BF16, tag="prod")
            nc.vector.tensor_tensor(
                out=prod[:], in0=aals[:], in1=gsl[:, :, d : 2 * d],
                op=mybir.AluOpType.mult,
            )
            dot = dvep.tile([P, HB], F32, tag="dot")
            red_eng = nc.gpsimd if REDUCE_ON_POOL else nc.vector
            red_eng.tensor_reduce(
                out=dot[:], in_=prod[:], op=mybir.AluOpType.add,
                axis=mybir.AxisListType.X,
            )
            # wv = (dot + 21) * (0.05 * val)  == val * (1.05 + 0.05*dot)
            wv = dvep.tile([P, HB], BF16, tag="wv")
            nc.vector.scalar_tensor_tensor(
                out=wv[:], in0=dot[:], scalar=21.0,
                in1=val_t[:, t0 : t0 + HB],
                op0=mybir.AluOpType.add, op1=mybir.AluOpType.mult,
            )
            msgs = msgp.tile([P, HB, d], BF16, tag="msgs")
            nc.vector.tensor_tensor(
                out=msgs[:], in0=gsl[:, :, 0:d],
                in1=wv[:].to_broadcast([P, HB, d]),
                op=mybir.AluOpType.mult,
            )
            # scatter matmuls
            for sl in range(HB):
                t = t0 + sl
                pss_t, w, j, first, last = tw[t]
                key = (pss_t, w)
                if key not in win_psum:
                    win_psum[key] = paccp.tile(
                        [P, d], F32, space="PSUM", tag="pacc", name="pacc"
                    )
                pw = win_psum[key]
                nc.tensor.matmul(
                    out=pw[:],
                    lhsT=s8[:, sl * P : (sl + 1) * P],
                    rhs=msgs[:, sl, :],
                    start=first,
                    stop=last,
                )
                if last:
                    if pss_t == 0:
                        if sched.T[1, w] == 0:
                            # no high-pass tiles: window is done now
                            close_window(w, pw[:], None, False)
                        else:
                            nc.scalar.copy(
                                out=acc[:, w * d : (w + 1) * d], in_=pw[:]
                            )
                    else:
                        has_low = sched.T[0, w] > 0
                        close_window(w, pw[:], acc[:, w * d : (w + 1) * d], has_low)
                    del win_psum[key]
        if c + LOOKAHEAD < ncalls:
            issue_gather(c + LOOKAHEAD)


def build_kernel(cfg: Config, sched1: StageSched, sched2: StageSched):
    nc = bacc.Bacc(
        "TRN2",
        target_bir_lowering=False,
        debug=False,
        enable_asserts=False,
        num_devices=cfg.n_cores,
        num_swdge_queues=4,
    )
    d = cfg.d
    nw = cfg.nw
    aug_in = nc.dram_tensor(
        "aug_full", [cfg.n_pad, 2 * d], BF16, kind="ExternalInput"
    )
    strip8_in = nc.dram_tensor("strip8", [P, nw * d], F8, kind="ExternalInput")
    stripbf_in = nc.dram_tensor(
        "stripbf", [P, nw * d], BF16, kind="ExternalInput"
    )
    sio = {}
    for s, sch in (("s1", sched1), ("s2", sched2)):
        nt = sch.total_tiles
        sio[s] = {
            "idx": nc.dram_tensor(f"{s}_idx", [P, (nt // TB) * (TB * P // 16)], I16, kind="ExternalInput"),
            "val": nc.dram_tensor(f"{s}_val", [P, nt], F32, kind="ExternalInput"),
            "dloc": nc.dram_tensor(f"{s}_dloc", [P, nt], BF16, kind="ExternalInput"),
            "spr": nc.dram_tensor(f"{s}_spr", [P, nt * P], F8, kind="ExternalInput"),
        }
    # stage-2 output, partition-major [p, w, d] (host re-assembles)
    out = nc.dram_tensor("out", [P, nw * d], F32, kind="ExternalOutput")

    with tile.TileContext(nc) as tc:
        cpool = tc.alloc_tile_pool(name="const", bufs=1)
        dram = tc.alloc_tile_pool(name="dram", bufs=1, space="DRAM")
        # constants
        iota_i = cpool.tile([P, P], I32)
        nc.gpsimd.iota(iota_i[:], pattern=[[1, P]], base=0, channel_multiplier=0)
        iota_bf = cpool.tile([P, P], BF16)
        nc.vector.tensor_copy(out=iota_bf[:], in_=iota_i[:])

        def iota_mid(tb):
            a = iota_bf[:]
            return bass.AP(a.tensor, a.offset, [a.ap[0], [0, tb], a.ap[1]])

        strip = cpool.tile([P, nw * d], F8)
        nc.sync.dma_start(strip[:], strip8_in[:, :])
        strip_bf = cpool.tile([P, nw * d], BF16)
        nc.sync.dma_start(strip_bf[:], stripbf_in[:, :])

        # ---------------- stage pools ----------------
        pools = {
            "g": tc.alloc_tile_pool(name="g", bufs=7),
            "aal": tc.alloc_tile_pool(name="aal", bufs=2, space="PSUM"),
            "pacc": tc.alloc_tile_pool(name="pacc", bufs=3, space="PSUM"),
            "spr": tc.alloc_tile_pool(name="sprp", bufs=3),
            "dve": tc.alloc_tile_pool(name="dve", bufs=5),
            "msg": tc.alloc_tile_pool(name="msg", bufs=5),
            "s8": tc.alloc_tile_pool(name="s8", bufs=5),
            "acc": tc.alloc_tile_pool(name="accp", bufs=1),
            "io": tc.alloc_tile_pool(name="iop", bufs=1),
            "stg": tc.alloc_tile_pool(name="stg", bufs=1),
        }
        consts = {"strip": strip, "iota_mid": iota_mid}

        # inter-hop tables split into window groups A (w < WA) and B so the
        # first AllGather overlaps the stage-1 tail and the second overlaps
        # stage-2's pass-A compute
        WA = nw // 2
        nwA, nwB = WA, nw - WA
        mlocA = dram.tile([P, nwA, 2 * d], BF16)
        mlocB = dram.tile([P, nwB, 2 * d], BF16)
        mfullA = dram.tile(
            [cfg.n_cores * P * nwA, 2 * d], BF16, addr_space="Shared"
        )
        mfullB = dram.tile(
            [cfg.n_cores * P * nwB, 2 * d], BF16, addr_space="Shared"
        )

        def load_stage_meta(s):
            # chunked loads: the first gathers only need the head of idx/dloc,
            # so fine-grained DMAs let compute start before the tail arrives
            io = pools["io"]
            t = sio[s]
            nt = t["val"].shape[1]
            nm = 4
            idx_t = io.tile([P, t["idx"].shape[1]], I16, tag=f"{s}i", name=f"{s}i")
            val_t = io.tile([P, nt], F32, tag=f"{s}v", name=f"{s}v")
            dloc_t = io.tile([P, nt], BF16, tag=f"{s}d", name=f"{s}d")
            ic = t["idx"].shape[1] // nm
            vc = nt // nm
            for m in range(nm):
                nc.sync.dma_start(
                    idx_t[:, m * ic : (m + 1) * ic], t["idx"][:, m * ic : (m + 1) * ic]
                )
                nc.sync.dma_start(
                    dloc_t[:, m * vc : (m + 1) * vc], t["dloc"][:, m * vc : (m + 1) * vc]
                )
                nc.sync.dma_start(
                    val_t[:, m * vc : (m + 1) * vc], t["val"][:, m * vc : (m + 1) * vc]
                )
            return idx_t, val_t, dloc_t, t["spr"]

        # ---------------- stage 1 (tar) ----------------
        idx1, val1, dloc1, sprd1 = load_stage_meta("s1")

        # staging tiles for stage-1 output [p, w, 2d]; norm halves prefilled
        stgA = pools["stg"].tile([P, nwA, 2 * d], BF16, tag="stgA")
        stgB = pools["stg"].tile([P, nwB, 2 * d], BF16, tag="stgB")
        sbf = strip_bf[:]
        nc.vector.tensor_copy(
            out=stgA[:, :, d : 2 * d],
            in_=bass.AP(sbf.tensor, sbf.offset, [sbf.ap[0], [d, nwA], [1, d]]),
        )
        nc.vector.tensor_copy(
            out=stgB[:, :, d : 2 * d],
            in_=bass.AP(
                sbf.tensor, sbf.offset + WA * d, [sbf.ap[0], [d, nwB], [1, d]]
            ),
        )

        def close1(w, psum_ap, acc_ap, has_low):
            tgt = (
                stgA[:, w, 0:d] if w < WA else stgB[:, w - WA, 0:d]
            )
            if psum_ap is None:
                nc.scalar.copy(out=tgt, in_=acc_ap)
            elif has_low:
                nc.vector.tensor_tensor(
                    out=tgt, in0=acc_ap, in1=psum_ap, op=mybir.AluOpType.add
                )
            else:
                nc.scalar.copy(out=tgt, in_=psum_ap)

        rg = [list(range(cfg.n_cores))]
        pend = {"A": nwA, "B": nwB, "agA": False, "agB": False}

        def emit_agA():
            nc.sync.dma_start(out=mlocA[:], in_=stgA[:])
            nc.gpsimd.collective_compute(
                "AllGather", mybir.AluOpType.bypass, replica_groups=rg,
                ins=[mlocA[:].opt()], outs=[mfullA[:].opt()],
            )
            pend["agA"] = True

        def emit_agB():
            nc.sync.dma_start(out=mlocB[:], in_=stgB[:])
            nc.gpsimd.collective_compute(
                "AllGather", mybir.AluOpType.bypass, replica_groups=rg,
                ins=[mlocB[:].opt()], outs=[mfullB[:].opt()],
            )
            pend["agB"] = True

        def on_close1(w):
            pend["A" if w < WA else "B"] -= 1
            if pend["A"] == 0 and not pend["agA"]:
                emit_agA()
            if pend["B"] == 0 and pend["agA"] and not pend["agB"]:
                emit_agB()

        _emit_stage(
            tc, cfg, sched1, pools, consts,
            (aug_in[0 : cfg.split, :], aug_in[cfg.split : cfg.n_pad, :]),
            idx1, val1, dloc1, sprd1, close1, on_window_close=on_close1,
        )
        assert pend["agA"] and pend["agB"]

        # ---------------- stage 2 (src) ----------------
        idx2, val2, dloc2, sprd2 = load_stage_meta("s2")

        stg2 = pools["stg"].tile([P, nw, d], F32, tag="stg2")

        def close2(w, psum_ap, acc_ap, has_low):
            tgt = stg2[:, w, :]
            if psum_ap is None:
                nc.scalar.copy(out=tgt, in_=acc_ap)
            elif has_low:
                nc.vector.tensor_tensor(
                    out=tgt, in0=acc_ap, in1=psum_ap, op=mybir.AluOpType.add
                )
            else:
                nc.scalar.copy(out=tgt, in_=psum_ap)

        _emit_stage(
            tc, cfg, sched2, pools, consts, (mfullA[:], mfullB[:]),
            idx2, val2, dloc2, sprd2, close2,
        )
        nc.sync.dma_start(out=out[:, :], in_=stg2[:].rearrange("p w f -> p (w f)"))

        for p in reversed(list(pools.values())):
            p.release()
        dram.release()
        cpool.release()

    nc.compile()
    return nc


def _host_tables(cfg: Config, embs):
    """Host-precomputed [raw | normalized] bf16 table + per-core strips."""
    embs_pad = np.zeros((cfg.n_pad, cfg.d), dtype=np.float32)
    embs_pad[: cfg.n_nodes] = np.asarray(embs, dtype=np.float32)
    n2 = np.sum(embs_pad * embs_pad, axis=1, keepdims=True)
    invn = 1.0 / np.sqrt(n2 + 1e-12)
    norm = (embs_pad * invn).astype(NP_BF16)
    aug = np.concatenate(
        [embs_pad.astype(NP_BF16), norm], axis=1
    )  # [n_pad, 2d]
    strips8, stripsbf = [], []
    for k in range(cfg.n_cores):
        blk = norm[k * cfg.rpc : (k + 1) * cfg.rpc]  # [rpc, d]
        # strip[p, w*d + j] = norm[w*128 + p, j]
        s = np.transpose(
            blk.reshape(cfg.nw, P, cfg.d), (1, 0, 2)
        ).reshape(P, cfg.nw * cfg.d)
        stripsbf.append(np.ascontiguousarray(s))
        strips8.append(s.astype(NP_F8))
    return aug, strips8, stripsbf


def _stage2_maps(cfg: Config):
    """The inter-hop tables are built from per-core [p, w, f] staging blocks
    split into window groups A (w < WA) and B: original row k*rpc + w*128 + p
    lands in table hi=(w>=WA) at row k*128*nwX + p*nwX + (w - base)."""
    WA = cfg.nw // 2
    nwA, nwB = WA, cfg.nw - WA
    r = np.arange(cfg.n_pad, dtype=np.int64)
    k, loc = r // cfg.rpc, r % cfg.rpc
    w, p = loc // P, loc % P
    hi = (w >= WA).astype(np.int64)
    colA = k * (P * nwA) + p * nwA + w
    colB = k * (P * nwB) + p * nwB + (w - WA)
    return hi, np.where(hi == 0, colA, colB)


def prepare(cfg: Config, inputs):
    """inputs: dict with pois_embs, src_edge_index, src_edge_val, tar_*."""
    sched1, meta1 = route_edges(cfg, inputs["tar_edge_index"], inputs["tar_edge_val"])
    hi2, col2 = _stage2_maps(cfg)
    sched2, meta2 = route_edges(
        cfg, inputs["src_edge_index"], inputs["src_edge_val"],
        hi_map=hi2, col_map=col2,
    )
    aug, strips8, stripsbf = _host_tables(cfg, inputs["pois_embs"])
    in_maps = []
    for k in range(cfg.n_cores):
        in_maps.append(
            {
                "aug_full": aug,
                "strip8": strips8[k],
                "stripbf": stripsbf[k],
                "s1_idx": meta1[k]["idx"], "s1_val": meta1[k]["val"],
                "s1_dloc": meta1[k]["dloc"], "s1_spr": meta1[k]["spr"],
                "s2_idx": meta2[k]["idx"], "s2_val": meta2[k]["val"],
                "s2_dloc": meta2[k]["dloc"], "s2_spr": meta2[k]["spr"],
            }
        )
    return sched1, sched2, in_maps


def assemble_output(cfg: Config, results):
    out = np.zeros((cfg.n_nodes, cfg.d), dtype=np.float32)
    for k, r in enumerate(results):
        # device layout [p, w*d] -> rows [w*128+p, d]
        blk = r["out"].reshape(P, cfg.nw, cfg.d).transpose(1, 0, 2)
        blk = blk.reshape(cfg.rpc, cfg.d)
        lo = k * cfg.rpc
        hi = min(lo + cfg.rpc, cfg.n_nodes)
        if hi > lo:
            out[lo:hi] = blk[0 : hi - lo]
    return out


_CACHE = {}


def kernel(**inputs):
    import concourse.bass_utils as bass_utils

    cfg = Config()
    sched1, sched2, in_maps = prepare(cfg, inputs)
    key = (sched1.n_tiles, sched2.n_tiles, tuple(sched1.T.ravel()), tuple(sched2.T.ravel()))
    nc = _CACHE.get(key)
    if nc is None:
        nc = build_kernel(cfg, sched1, sched2)
        _CACHE[key] = nc
    res = bass_utils.run_bass_kernel_spmd(
        nc, in_maps, core_ids=list(range(cfg.n_cores)), trace=False
    )
    out = assemble_output(cfg, res.results)
    return out.astype(np.float32, copy=False)


# revision 35
# speedup vs baseline: 1.6042x; 1.6042x over previous
"""Trainium2 Bass kernel for nn_DirectedHyperConvLayer (GNN message passing).

Self-contained: accepts FULL inputs, shards across 8 NeuronCores internally,
returns the FULL [50000, 64] float32 output.

Sharding: each core owns a contiguous block of destination rows; the host
routes/sorts edges by destination row (and by source row within each group
for gather locality), pads them into 128-edge tiles grouped by 128-row
destination windows, and splits each window's edges into low/high passes so
source indices fit dma_gather's int16 limit. On device, combined
[raw_bf16 | normalized_bf16] rows are fetched with batched dma_gather calls;
per-edge cosine weights use a host-supplied f8 one-hot selecting the
destination-window embedding block through the tensor engine; weighted
messages are segment-summed by one-hot matmuls into PSUM per window.

v2: the normalized-embedding table (aug) is host-precomputed and staged as a
replicated input (no prologue normalize, no first AllGather); spr one-hots
stream in a few large chunked DMAs instead of per-window transfers; stage
outputs accumulate in an SBUF staging tile (partition-major layout) and are
written with one large DMA per stage; only the inter-hop AllGather remains.
"""


import math
from dataclasses import dataclass

import numpy as np
import ml_dtypes

import concourse.bass as bass
import concourse.bacc as bacc
import concourse.mybir as mybir
import concourse.tile as tile

F32 = mybir.dt.float32
BF16 = mybir.dt.bfloat16
F8 = mybir.dt.float8e4
I16 = mybir.dt.int16
I32 = mybir.dt.int32
NP_F8 = mybir.dt.np(F8)
NP_BF16 = mybir.dt.np(BF16)
P = 128
TB = 16  # tiles per compute batch
TBG = 8  # tiles per gather sub-call (1024 idx — proven ucode granularity)
HB = 16  # tiles per compute half-batch (must divide TB)
CT = 112  # spr tiles per streamed chunk
ALPHA = 0.1
REDUCE_ON_POOL = False  # gpsimd can't do free-axis reduce (C-axis only)


@dataclass
class Config:
    n_nodes: int = 50000
    d: int = 64
    n_cores: int = 8
    rpc: int = 6272  # rows per core (multiple of 128)
    split: int = 32768  # low/high gather split (<= 32768)

    @property
    def nw(self):
        return self.rpc // P

    @property
    def n_pad(self):
        return self.n_cores * self.rpc

    @property
    def nblk(self):
        return self.n_pad // P


@dataclass
class StageSched:
    T: np.ndarray  # [2, nw] tiles per (pass, window)
    n_tiles: tuple  # (low, high) tile counts (each % TB == 0)

    @property
    def total_tiles(self):
        return int(self.n_tiles[0] + self.n_tiles[1])

    def tile_windows(self):
        """list over global tile index -> (pass, w, j_in_window, first, last)"""
        out = []
        for p in range(2):
            for w in range(self.T.shape[1]):
                Tw = int(self.T[p, w])
                for j in range(Tw):
                    out.append((p, w, j, j == 0, j == Tw - 1))
        return out


def route_edges(cfg: Config, edge_index, edge_val, hi_map=None, col_map=None):
    """Returns (sched, per_core list of dicts with idx16/val05/dloc/spr).

    hi_map/col_map: optional [n_pad] arrays giving, per original source row,
    which gather table it lives in (0/1) and its row index there. Defaults
    describe the stage-1 aug table split at cfg.split."""
    r0 = np.asarray(edge_index[0], dtype=np.int64)
    r1 = np.asarray(edge_index[1], dtype=np.int64)
    val = np.asarray(edge_val, dtype=np.float32)
    E = r0.shape[0]
    nc_, nw = cfg.n_cores, cfg.nw

    k = r0 // cfg.rpc
    w = (r0 % cfg.rpc) // P
    dloc = r0 % P
    if hi_map is None:
        hi = (r1 >= cfg.split).astype(np.int64)
        col = r1 - hi * cfg.split
    else:
        hi = hi_map[r1]
        col = col_map[r1]
    gid = (k * 2 + hi) * nw + w

    counts = np.bincount(gid, minlength=nc_ * 2 * nw).reshape(nc_, 2, nw)
    T = np.ceil(counts.max(axis=0) / P).astype(np.int64)  # [2, nw]
    # pad each pass's tile total to a multiple of TB
    for pss in range(2):
        T[pss, nw - 1] += (-int(T[pss].sum())) % TB
    nt_low, nt_high = int(T[0].sum()), int(T[1].sum())
    n_tiles = nt_low + nt_high
    # global tile base per (pass, w)
    tbase = np.zeros((2, nw), dtype=np.int64)
    tbase[0] = np.cumsum(T[0]) - T[0]
    tbase[1] = nt_low + np.cumsum(T[1]) - T[1]

    # slot within (k, hi, w) group; edges sorted by source row within each
    # group so gather descriptors hit ascending HBM addresses
    order = np.argsort(gid * (1 << 17) + col, kind="stable")
    sorted_gid = gid[order]
    starts = np.searchsorted(sorted_gid, np.arange(nc_ * 2 * nw))
    ranks = np.empty(E, dtype=np.int64)
    ranks[order] = np.arange(E) - starts[sorted_gid]

    tile_g = tbase[hi, w] + ranks // P  # global tile per edge
    pos = ranks % P
    idx_val = col.astype(np.int16)

    per_core = []
    for kk in range(nc_):
        m = k == kk
        tg, pg = tile_g[m], pos[m]
        slots = tg * P + pg
        # padding slots re-gather the previous real row (HBM row-buffer hit,
        # keeps the call's addresses ascending; mid-call negative indices
        # crash the gather ucode); val=0 and dloc=-1 zero their contribution
        idx_flat = np.zeros(n_tiles * P, dtype=np.int16)
        val_flat = np.zeros(n_tiles * P, dtype=np.float32)
        dloc_flat = np.full(n_tiles * P, -1.0, dtype=np.float32)
        idx_flat[slots] = idx_val[m] + 1  # 1-based so ffill can spot pads
        val_flat[slots] = 0.05 * val[m]
        dloc_flat[slots] = dloc[m].astype(np.float32)
        nz = np.where(idx_flat != 0)[0]
        if nz.size:
            ff = np.maximum.accumulate(
                np.where(idx_flat != 0, np.arange(n_tiles * P), -1)
            )
            filled = np.where(ff >= 0, idx_flat[np.maximum(ff, 0)], 1)
            idx_flat = (filled - 1).astype(np.int16)
        else:
            idx_flat[:] = 0
        # idx16 wrapped per 1024-idx gather sub-call: [128, (n_tiles//TBG)*cwg]
        cwg = TBG * P // 16
        ncalls_g = n_tiles // TBG
        iw = idx_flat.reshape(ncalls_g, cwg, 16)
        iw = np.transpose(iw, (2, 0, 1)).reshape(16, ncalls_g * cwg)
        idx16 = np.tile(iw, (8, 1))
        # val/dloc [128, n_tiles] (partition p, tile t)
        val2d = val_flat.reshape(n_tiles, P).T.copy()
        dloc2d = dloc_flat.reshape(n_tiles, P).T.astype(NP_BF16)
        # S' [128(d), n_tiles*128(e)] f8
        spr = np.zeros((P, n_tiles * P), dtype=NP_F8)
        spr[dloc[m], slots] = NP_F8(1.0)
        per_core.append(
            {"idx": idx16, "val": val2d, "dloc": dloc2d, "spr": spr}
        )
    return StageSched(T=T, n_tiles=(nt_low, nt_high)), per_core


def _emit_stage(
    tc, cfg, sched: StageSched, pools, consts, tables, idx_t, val_t, dloc_t,
    spr_dram, out_close, on_window_close=None, mid_hooks=None,
):
    """Emit one spmm stage. out_close(w, psum_ap, acc_ap, has_low) writes the
    finished window; on_window_close(w) fires after each window close;
    mid_hooks[c] is invoked at the top of compute batch c."""
    nc = tc.nc
    nw = cfg.nw
    d = cfg.d
    gp, aalp, paccp, dvep, msgp, s8p, accp, sprp = (
        pools["g"], pools["aal"], pools["pacc"], pools["dve"], pools["msg"],
        pools["s8"], pools["acc"], pools["spr"],
    )
    strip = consts["strip"]
    iota_mid = consts["iota_mid"]

    acc = accp.tile([P, nw * d], F32, tag="acc")
    nc.vector.memset(acc[:], 0.0)

    def close_window(w, psum_ap, acc_ap, has_low):
        out_close(w, psum_ap, acc_ap, has_low)
        if on_window_close is not None:
            on_window_close(w)

    # windows with no edges at all: close immediately from the zeroed acc
    for w in range(nw):
        if sched.T[0, w] == 0 and sched.T[1, w] == 0:
            close_window(w, None, acc[:, w * d : (w + 1) * d], False)

    tw = sched.tile_windows()
    n_tiles = sched.total_tiles
    assert n_tiles % TB == 0
    ncalls = n_tiles // TB
    nt_low = sched.n_tiles[0]
    cwg = TBG * P // 16  # idx cols per gather sub-call
    nsub = TB // TBG  # gather sub-calls per compute batch

    # spr streamed in CT-tile chunks (few large DMAs, rolling triple-buffer)
    nchunks = (n_tiles + CT - 1) // CT
    spr_chunks = {}
    next_chunk = [0]

    def issue_chunk():
        ci = next_chunk[0]
        t0 = ci * CT
        ctn = min(CT, n_tiles - t0)
        st = sprp.tile([P, CT * P], F8, tag="spr", name="spr")
        nc.scalar.dma_start(st[:, 0 : ctn * P], spr_dram[:, t0 * P : (t0 + ctn) * P])
        spr_chunks[ci] = st
        next_chunk[0] += 1

    while next_chunk[0] < min(2, nchunks):
        issue_chunk()

    win_psum = {}
    g_tiles = {}
    # exactly 2 outstanding gather calls per SWDGE queue (8 sub-calls / 4
    # queues): the ring holds no more — deeper lookahead crashes the ucode.
    LOOKAHEAD = 4

    gseen = consts.setdefault("gseen", [0])

    def issue_gather(c):
        pss = 0 if c * TB < nt_low else 1
        tab = tables[pss]
        g = gp.tile([P, TB, 2 * d], BF16, tag="g")
        if gseen[0] < gp.bufs:
            # first touch of this ring buffer: clear it so slots skipped by
            # negative gather indices hold finite values, not random bits
            nc.vector.memset(g[:], 0.0)
            gseen[0] += 1
        for sub in range(nsub):
            sc = c * nsub + sub  # global gather sub-call index
            nc.gpsimd.dma_gather(
                out_ap=g[:, sub * TBG : (sub + 1) * TBG, :],
                in_ap=tab,
                idxs_ap=idx_t[:, sc * cwg : (sc + 1) * cwg],
                num_idxs=TBG * P,
                num_idxs_reg=TBG * P,
                elem_size=2 * d,
                queue_num=sc % 4,
                single_packet=True,
            )
        g_tiles[c] = g

    for c in range(min(LOOKAHEAD, ncalls)):
        issue_gather(c)
    for c in range(ncalls):
        if mid_hooks and c in mid_hooks:
            mid_hooks[c]()
        # keep 2 spr chunks of lookahead in flight
        while next_chunk[0] < nchunks and next_chunk[0] * CT < (c + 3) * TB:
            issue_chunk()
        g = g_tiles.pop(c)
        for half in range(TB // HB):
            h = c * (TB // HB) + half  # half-batch index
            t0 = c * TB + half * HB  # first tile of this half-batch
            # one-hot S8 [128(e), HB, 128(d)] in bf16 — emitted first so DVE
            # computes it while the batch's gather is still in flight
            s8 = s8p.tile([P, HB * P], BF16, tag="s8")
            s8v = bass.AP(
                s8[:].tensor, s8[:].offset, [s8[:].ap[0], [P, HB], [1, P]]
            )
            nc.vector.tensor_tensor(
                out=s8v,
                in0=dloc_t[:, t0 : t0 + HB].to_broadcast([P, HB, P]),
                in1=iota_mid(HB),
                op=mybir.AluOpType.is_equal,
            )
            # Aal matmuls (per tile)
            aal = aalp.tile([P, HB, d], F32, space="PSUM", tag="aal")
            for sl in range(HB):
                t = t0 + sl
                pss_t, w, _, _, _ = tw[t]
                st = spr_chunks[t // CT]
                jj = t % CT
                nc.tensor.matmul(
                    out=aal[:, sl, :],
                    lhsT=st[:, jj * P : (jj + 1) * P],
                    rhs=strip[:, w * d : (w + 1) * d],
                    start=True,
                    stop=True,
                )
            gsl = g[:, half * HB : (half + 1) * HB, :]
            # drain aal PSUM -> SBUF bf16 on ACT so DVE runs in 16-bit mode
            aals = dvep.tile([P, HB, d], BF16, tag="aals")
            nc.scalar.copy(out=aals[:], in_=aal[:])
            # dot / weight / msgs (batched over HB tiles)
            prod = dvep.tile([P, HB, d], BF16, tag="prod")
            nc.vector.tensor_tensor(
                out=prod[:], in0=aals[:], in1=gsl[:, :, d : 2 * d],
                op=mybir.AluOpType.mult,
            )
            dot = dvep.tile([P, HB], F32, tag="dot")
            red_eng = nc.gpsimd if REDUCE_ON_POOL else nc.vector
            red_eng.tensor_reduce(
                out=dot[:], in_=prod[:], op=mybir.AluOpType.add,
                axis=mybir.AxisListType.X,
            )
            # wv = (dot + 21) * (0.05 * val)  == val * (1.05 + 0.05*dot)
            wv = dvep.tile([P, HB], BF16, tag="wv")
            nc.vector.scalar_tensor_tensor(
                out=wv[:], in0=dot[:], scalar=21.0,
                in1=val_t[:, t0 : t0 + HB],
                op0=mybir.AluOpType.add, op1=mybir.AluOpType.mult,
            )
            msgs = msgp.tile([P, HB, d], BF16, tag="msgs")
            nc.vector.tensor_tensor(
                out=msgs[:], in0=gsl[:, :, 0:d],
                in1=wv[:].to_broadcast([P, HB, d]),
                op=mybir.AluOpType.mult,
            )
            # scatter matmuls
            for sl in range(HB):
                t = t0 + sl
                pss_t, w, j, first, last = tw[t]
                key = (pss_t, w)
                if key not in win_psum:
                    win_psum[key] = paccp.tile(
                        [P, d], F32, space="PSUM", tag="pacc", name="pacc"
                    )
                pw = win_psum[key]
                nc.tensor.matmul(
                    out=pw[:],
                    lhsT=s8[:, sl * P : (sl + 1) * P],
                    rhs=msgs[:, sl, :],
                    start=first,
                    stop=last,
                )
                if last:
                    if pss_t == 0:
                        if sched.T[1, w] == 0:
                            # no high-pass tiles: window is done now
                            close_window(w, pw[:], None, False)
                        else:
                            nc.scalar.copy(
                                out=acc[:, w * d : (w + 1) * d], in_=pw[:]
                            )
                    else:
                        has_low = sched.T[0, w] > 0
                        close_window(w, pw[:], acc[:, w * d : (w + 1) * d], has_low)
                    del win_psum[key]
        if c + LOOKAHEAD < ncalls:
            issue_gather(c + LOOKAHEAD)


def build_kernel(cfg: Config, sched1: StageSched, sched2: StageSched):
    nc = bacc.Bacc(
        "TRN2",
        target_bir_lowering=False,
        debug=False,
        enable_asserts=False,
        num_devices=cfg.n_cores,
        num_swdge_queues=4,
    )
    d = cfg.d
    nw = cfg.nw
    aug_in = nc.dram_tensor(
        "aug_full", [cfg.n_pad, 2 * d], BF16, kind="ExternalInput"
    )
    strip8_in = nc.dram_tensor("strip8", [P, nw * d], F8, kind="ExternalInput")
    stripbf_in = nc.dram_tensor(
        "stripbf", [P, nw * d], BF16, kind="ExternalInput"
    )
    sio = {}
    for s, sch in (("s1", sched1), ("s2", sched2)):
        nt = sch.total_tiles
        sio[s] = {
            "idx": nc.dram_tensor(f"{s}_idx", [P, (nt // TB) * (TB * P // 16)], I16, kind="ExternalInput"),
            "val": nc.dram_tensor(f"{s}_val", [P, nt], F32, kind="ExternalInput"),
            "dloc": nc.dram_tensor(f"{s}_dloc", [P, nt], BF16, kind="ExternalInput"),
            "spr": nc.dram_tensor(f"{s}_spr", [P, nt * P], F8, kind="ExternalInput"),
        }
    # stage-2 output, partition-major [p, w, d] (host re-assembles)
    out = nc.dram_tensor("out", [P, nw * d], F32, kind="ExternalOutput")

    with tile.TileContext(nc) as tc:
        cpool = tc.alloc_tile_pool(name="const", bufs=1)
        dram = tc.alloc_tile_pool(name="dram", bufs=1, space="DRAM")
        # constants
        iota_i = cpool.tile([P, P], I32)
        nc.gpsimd.iota(iota_i[:], pattern=[[1, P]], base=0, channel_multiplier=0)
        iota_bf = cpool.tile([P, P], BF16)
        nc.vector.tensor_copy(out=iota_bf[:], in_=iota_i[:])

        def iota_mid(tb):
            a = iota_bf[:]
            return bass.AP(a.tensor, a.offset, [a.ap[0], [0, tb], a.ap[1]])

        strip = cpool.tile([P, nw * d], F8)
        nc.sync.dma_start(strip[:], strip8_in[:, :])
        strip_bf = cpool.tile([P, nw * d], BF16)
        nc.sync.dma_start(strip_bf[:], stripbf_in[:, :])

        # ---------------- stage pools ----------------
        pools = {
            "g": tc.alloc_tile_pool(name="g", bufs=7),
            "aal": tc.alloc_tile_pool(name="aal", bufs=2, space="PSUM"),
            "pacc": tc.alloc_tile_pool(name="pacc", bufs=3, space="PSUM"),
            "spr": tc.alloc_tile_pool(name="sprp", bufs=3),
            "dve": tc.alloc_tile_pool(name="dve", bufs=5),
            "msg": tc.alloc_tile_pool(name="msg", bufs=5),
            "s8": tc.alloc_tile_pool(name="s8", bufs=5),
            "acc": tc.alloc_tile_pool(name="accp", bufs=1),
            "io": tc.alloc_tile_pool(name="iop", bufs=1),
            "stg": tc.alloc_tile_pool(name="stg", bufs=1),
        }
        consts = {"strip": strip, "iota_mid": iota_mid}

        # inter-hop tables split into window groups A (w < WA) and B so the
        # first AllGather overlaps the stage-1 tail and the second overlaps
        # stage-2's pass-A compute
        WA = nw // 2
        nwA, nwB = WA, nw - WA
        mlocA = dram.tile([P, nwA, 2 * d], BF16)
        mlocB = dram.tile([P, nwB, 2 * d], BF16)
        mfullA = dram.tile(
            [cfg.n_cores * P * nwA, 2 * d], BF16, addr_space="Shared"
        )
        mfullB = dram.tile(
            [cfg.n_cores * P * nwB, 2 * d], BF16, addr_space="Shared"
        )

        def load_stage_meta(s):
            # chunked loads: the first gathers only need the head of idx/dloc,
            # so fine-grained DMAs let compute start before the tail arrives
            io = pools["io"]
            t = sio[s]
            nt = t["val"].shape[1]
            nm = 4
            idx_t = io.tile([P, t["idx"].shape[1]], I16, tag=f"{s}i", name=f"{s}i")
            val_t = io.tile([P, nt], F32, tag=f"{s}v", name=f"{s}v")
            dloc_t = io.tile([P, nt], BF16, tag=f"{s}d", name=f"{s}d")
            ic = t["idx"].shape[1] // nm
            vc = nt // nm
            for m in range(nm):
                nc.sync.dma_start(
                    idx_t[:, m * ic : (m + 1) * ic], t["idx"][:, m * ic : (m + 1) * ic]
                )
                nc.sync.dma_start(
                    dloc_t[:, m * vc : (m + 1) * vc], t["dloc"][:, m * vc : (m + 1) * vc]
                )
                nc.sync.dma_start(
                    val_t[:, m * vc : (m + 1) * vc], t["val"][:, m * vc : (m + 1) * vc]
                )
            return idx_t, val_t, dloc_t, t["spr"]

        # ---------------- stage 1 (tar) ----------------
        idx1, val1, dloc1, sprd1 = load_stage_meta("s1")

        # staging tiles for stage-1 output [p, w, 2d]; norm halves prefilled
        stgA = pools["stg"].tile([P, nwA, 2 * d], BF16, tag="stgA")
        stgB = pools["stg"].tile([P, nwB, 2 * d], BF16, tag="stgB")
        sbf = strip_bf[:]
        nc.vector.tensor_copy(
            out=stgA[:, :, d : 2 * d],
            in_=bass.AP(sbf.tensor, sbf.offset, [sbf.ap[0], [d, nwA], [1, d]]),
        )
        nc.vector.tensor_copy(
            out=stgB[:, :, d : 2 * d],
            in_=bass.AP(
                sbf.tensor, sbf.offset + WA * d, [sbf.ap[0], [d, nwB], [1, d]]
            ),
        )

        def close1(w, psum_ap, acc_ap, has_low):
            tgt = (
                stgA[:, w, 0:d] if w < WA else stgB[:, w - WA, 0:d]
            )
            if psum_ap is None:
                nc.scalar.copy(out=tgt, in_=acc_ap)
            elif has_low:
                nc.vector.tensor_tensor(
                    out=tgt, in0=acc_ap, in1=psum_ap, op=mybir.AluOpType.add
                )
            else:
                nc.scalar.copy(out=tgt, in_=psum_ap)

        rg = [list(range(cfg.n_cores))]
        pend = {"A": nwA, "B": nwB, "agA": False, "agB": False}

        def emit_agA():
            nc.sync.dma_start(out=mlocA[:], in_=stgA[:])
            nc.gpsimd.collective_compute(
                "AllGather", mybir.AluOpType.bypass, replica_groups=rg,
                ins=[mlocA[:].opt()], outs=[mfullA[:].opt()],
            )
            pend["agA"] = True

        def emit_agB():
            nc.sync.dma_start(out=mlocB[:], in_=stgB[:])
            nc.gpsimd.collective_compute(
                "AllGather", mybir.AluOpType.bypass, replica_groups=rg,
                ins=[mlocB[:].opt()], outs=[mfullB[:].opt()],
            )
            pend["agB"] = True

        def on_close1(w):
            pend["A" if w < WA else "B"] -= 1
            if pend["A"] == 0 and not pend["agA"]:
                emit_agA()
            if pend["B"] == 0 and pend["agA"] and not pend["agB"]:
                emit_agB()

        _emit_stage(
            tc, cfg, sched1, pools, consts,
            (aug_in[0 : cfg.split, :], aug_in[cfg.split : cfg.n_pad, :]),
            idx1, val1, dloc1, sprd1, close1, on_window_close=on_close1,
        )
        assert pend["agA"] and pend["agB"]

        # ---------------- stage 2 (src) ----------------
        idx2, val2, dloc2, sprd2 = load_stage_meta("s2")

        stg2 = pools["stg"].tile([P, nw, d], F32, tag="stg2")

        def close2(w, psum_ap, acc_ap, has_low):
            tgt = stg2[:, w, :]
            if psum_ap is None:
                nc.scalar.copy(out=tgt, in_=acc_ap)
            elif has_low:
                nc.vector.tensor_tensor(
                    out=tgt, in0=acc_ap, in1=psum_ap, op=mybir.AluOpType.add
                )
            else:
                nc.scalar.copy(out=tgt, in_=psum_ap)

        _emit_stage(
            tc, cfg, sched2, pools, consts, (mfullA[:], mfullB[:]),
            idx2, val2, dloc2, sprd2, close2,
        )
        nc.sync.dma_start(out=out[:, :], in_=stg2[:].rearrange("p w f -> p (w f)"))

        for p in reversed(list(pools.values())):
            p.release()
        dram.release()
        cpool.release()

    nc.compile()
    return nc


def _host_tables(cfg: Config, embs):
    """Host-precomputed [raw | normalized] bf16 table + per-core strips."""
    embs_pad = np.zeros((cfg.n_pad, cfg.d), dtype=np.float32)
    embs_pad[: cfg.n_nodes] = np.asarray(embs, dtype=np.float32)
    n2 = np.sum(embs_pad * embs_pad, axis=1, keepdims=True)
    invn = 1.0 / np.sqrt(n2 + 1e-12)
    norm = (embs_pad * invn).astype(NP_BF16)
    aug = np.concatenate(
        [embs_pad.astype(NP_BF16), norm], axis=1
    )  # [n_pad, 2d]
    strips8, stripsbf = [], []
    for k in range(cfg.n_cores):
        blk = norm[k * cfg.rpc : (k + 1) * cfg.rpc]  # [rpc, d]
        # strip[p, w*d + j] = norm[w*128 + p, j]
        s = np.transpose(
            blk.reshape(cfg.nw, P, cfg.d), (1, 0, 2)
        ).reshape(P, cfg.nw * cfg.d)
        stripsbf.append(np.ascontiguousarray(s))
        strips8.append(s.astype(NP_F8))
    return aug, strips8, stripsbf


def _stage2_maps(cfg: Config):
    """The inter-hop tables are built from per-core [p, w, f] staging blocks
    split into window groups A (w < WA) and B: original row k*rpc + w*128 + p
    lands in table hi=(w>=WA) at row k*128*nwX + p*nwX + (w - base)."""
    WA = cfg.nw // 2
    nwA, nwB = WA, cfg.nw - WA
    r = np.arange(cfg.n_pad, dtype=np.int64)
    k, loc = r // cfg.rpc, r % cfg.rpc
    w, p = loc // P, loc % P
    hi = (w >= WA).astype(np.int64)
    colA = k * (P * nwA) + p * nwA + w
    colB = k * (P * nwB) + p * nwB + (w - WA)
    return hi, np.where(hi == 0, colA, colB)


def prepare(cfg: Config, inputs):
    """inputs: dict with pois_embs, src_edge_index, src_edge_val, tar_*."""
    sched1, meta1 = route_edges(cfg, inputs["tar_edge_index"], inputs["tar_edge_val"])
    hi2, col2 = _stage2_maps(cfg)
    sched2, meta2 = route_edges(
        cfg, inputs["src_edge_index"], inputs["src_edge_val"],
        hi_map=hi2, col_map=col2,
    )
    aug, strips8, stripsbf = _host_tables(cfg, inputs["pois_embs"])
    in_maps = []
    for k in range(cfg.n_cores):
        in_maps.append(
            {
                "aug_full": aug,
                "strip8": strips8[k],
                "stripbf": stripsbf[k],
                "s1_idx": meta1[k]["idx"], "s1_val": meta1[k]["val"],
                "s1_dloc": meta1[k]["dloc"], "s1_spr": meta1[k]["spr"],
                "s2_idx": meta2[k]["idx"], "s2_val": meta2[k]["val"],
                "s2_dloc": meta2[k]["dloc"], "s2_spr": meta2[k]["spr"],
            }
        )
    return sched1, sched2, in_maps


def assemble_output(cfg: Config, results):
    out = np.zeros((cfg.n_nodes, cfg.d), dtype=np.float32)
    for k, r in enumerate(results):
        # device layout [p, w*d] -> rows [w*128+p, d]
        blk = r["out"].reshape(P, cfg.nw, cfg.d).transpose(1, 0, 2)
        blk = blk.reshape(cfg.rpc, cfg.d)
        lo = k * cfg.rpc
        hi = min(lo + cfg.rpc, cfg.n_nodes)
        if hi > lo:
            out[lo:hi] = blk[0 : hi - lo]
    return out


_CACHE = {}


def kernel(**inputs):
    import concourse.bass_utils as bass_utils

    cfg = Config()
    sched1, sched2, in_maps = prepare(cfg, inputs)
    key = (sched1.n_tiles, sched2.n_tiles, tuple(sched1.T.ravel()), tuple(sched2.T.ravel()))
    nc = _CACHE.get(key)
    if nc is None:
        nc = build_kernel(cfg, sched1, sched2)
        _CACHE[key] = nc
    res = bass_utils.run_bass_kernel_spmd(
        nc, in_maps, core_ids=list(range(cfg.n_cores)), trace=False
    )
    out = assemble_output(cfg, res.results)
    return out.astype(np.float32, copy=False)
